# revision 21
# baseline (speedup 1.0000x reference)
"""Exaone GQA flash-attention block on 8 Trainium2 NeuronCores.

Sharding: core pair (2p, 2p+1) handles prefill sequence p (S=1024). Within a
pair, the 8 causal 128-token q-chunks are split {0,2,5,7} / {1,3,4,6} so the
per-chunk key-block counts {1,3,6,8} vs {2,4,5,7} both fit under the uniform
compile-time schedule (2,4,6,8) with only 2 wasted blocks per core. No
cross-core communication: every core produces final output rows for its own
512 q-tokens; the host concatenates.

All layout work happens on the host (numpy, free): hs is transposed, weights
are rearranged into their exact SBUF layouts, everything is cast to bf16, and
rope tables / causal masks are prebuilt. The device program is pure DMA-in ->
matmul chains -> attention -> matmul chains -> DMA-out:

  phase A: kT = Wk^T @ hsT (rope via a +-1 rotation matmul), V natural
           [tok, ch] with an appended ones column so the PV matmul also
           produces the softmax denominator
  phase B (per kv-pair a): Q chains + rope for the 4 slots of group a, then
           attention: scores per (par, q-chunk, key-block) as two row-paired
           K=64 matmuls (partitions 0-63 / 64-127 run concurrently in
           different PE row groups), one fused exp over both parities on ACT,
           multiplicative masks only on the last two schedule positions,
           PV accumulated in PSUM [65, 4, 128]; normalization via the
           broadcast reciprocal of the ones-row
  phase C: out = attn^T.T @ Wo streamed per 512-wide output chunk.
"""
import sys
sys.path.insert(0, '/opt/trn_rl_repo')

from contextlib import ExitStack

import ml_dtypes
import numpy as np

import concourse.bass as bass
import concourse.mybir as mybir
import concourse.tile as tile
from concourse import bacc
from concourse.bass_utils import run_bass_kernel_spmd

F32 = mybir.dt.float32
BF16 = mybir.dt.bfloat16
AF = mybir.ActivationFunctionType
MUL = mybir.AluOpType.mult
ADD = mybir.AluOpType.add

B, S, D = 4, 1024, 2048
HQ, HKV, HD = 32, 8, 64
SCALE = HD ** -0.5
NQ = 512                       # q tokens per core
SCHED = (2, 4, 6, 8)           # key blocks per schedule slot (uniform)
CHUNKS_EVEN = (0, 2, 5, 7)     # q-chunk of schedule slot j, even cores
CHUNKS_ODD = (1, 3, 4, 6)


def build_nc():
    nc = bacc.Bacc("TRN2", target_bir_lowering=False, debug=False,
                   num_devices=8, num_swdge_queues=4)

    hsT_d = nc.dram_tensor("hsT", [128, 2, 16, 512], BF16, kind="ExternalInput")
    hsqT_d = nc.dram_tensor("hsqT", [128, 16, NQ], BF16, kind="ExternalInput")
    wk_d = nc.dram_tensor("wk", [128, 4, 16, 128], BF16, kind="ExternalInput")
    wv_d = nc.dram_tensor("wv", [128, 16, 512], BF16, kind="ExternalInput")
    wq_d = nc.dram_tensor("wq", [128, 4, 16, 4, 128], BF16, kind="ExternalInput")
    wo_d = nc.dram_tensor("wo", [128, 16, D], BF16, kind="ExternalInput")
    c4k_d = nc.dram_tensor("c4k", [128, S], BF16, kind="ExternalInput")
    s4k_d = nc.dram_tensor("s4k", [128, S], BF16, kind="ExternalInput")
    c4q_d = nc.dram_tensor("c4q", [128, NQ], BF16, kind="ExternalInput")
    s4q_d = nc.dram_tensor("s4q", [128, NQ], BF16, kind="ExternalInput")
    rot_d = nc.dram_tensor("rot", [128, 128], BF16, kind="ExternalInput")
    masks_d = nc.dram_tensor("masks", [128, 4, 2, 128], BF16, kind="ExternalInput")
    out = nc.dram_tensor("out", [NQ, D], F32, kind="ExternalOutput")

    with tile.TileContext(nc) as tc:
        with ExitStack() as ctx:
            pool = lambda *a, **k: ctx.enter_context(tc.tile_pool(*a, **k))
            qT_p = pool(name="qT", bufs=1)
            kT_p = pool(name="kT", bufs=1)
            v_p = pool(name="vsb", bufs=1)
            attn_p = pool(name="attn", bufs=1)
            const_p = pool(name="const", bufs=1)
            exp_p = pool(name="exps", bufs=4)
            rope_p = pool(name="rope", bufs=2)
            norm_p = pool(name="norm", bufs=1)

            qT = qT_p.tile([128, 16, NQ], BF16)
            kT = kT_p.tile([128, 4, S], BF16)
            v_sb = v_p.tile([128, 8, 8, 65], BF16)
            attn_sb = attn_p.tile([128, 16, NQ], BF16)

            rot_bf = const_p.tile([128, 128], BF16)
            masks_bf = const_p.tile([128, 4, 2, 128], BF16)
            c4k = const_p.tile([128, S], BF16)
            s4k = const_p.tile([128, S], BF16)
            c4q = const_p.tile([128, NQ], BF16)
            s4q = const_p.tile([128, NQ], BF16)

            nc.vector.memset(v_sb[:, :, :, 64], 1.0)

            # rope in two halves so the rot matmul never heads the PE queue
            # before its x_sb copy is ready: the finish part is emitted after
            # the NEXT chain's matmuls (deferred via `pending`).
            pending = []

            def flush():
                for f in pending:
                    f()
                pending.clear()

            def rope_defer(psum, c4, s4, col0, n, dst, psum_pool, tag):
                x_sb = rope_p.tile([128, n], BF16, tag="rsb")
                nc.vector.tensor_copy(x_sb[:], psum[:])

                def fin():
                    pr = psum_pool.tile([128, n], F32, tag=tag)
                    nc.tensor.matmul(pr[:], rot_bf[:], x_sb[:],
                                     start=True, stop=True)
                    t1 = rope_p.tile([128, n], BF16, tag="rt1")
                    nc.vector.tensor_tensor(t1[:], pr[:], s4[:, col0:col0 + n],
                                            MUL)
                    t2 = rope_p.tile([128, n], BF16, tag="rt2")
                    nc.vector.tensor_tensor(t2[:], x_sb[:], c4[:, col0:col0 + n],
                                            MUL)
                    nc.vector.tensor_tensor(dst, t1[:], t2[:], ADD)
                pending.append(fin)

            hsq_p = pool(name="hsq", bufs=1)
            wq_p = pool(name="wqbf", bufs=2)
            hsqT = hsq_p.tile([128, 16, NQ], BF16)
            wq_tiles = []

            # ---- phase A: K/V projections + Q chains for group 0 ----
            with ExitStack() as actx:
                apool = lambda *a, **k: actx.enter_context(tc.tile_pool(*a, **k))
                hsT_p = apool(name="hsT", bufs=1)
                wk_p = apool(name="wkbf", bufs=1)
                wv_p2 = apool(name="wvbf", bufs=1)
                projA = apool(name="projA", bufs=6, space="PSUM")

                hsT = hsT_p.tile([128, 2, 16, 512], BF16)
                wk_bf = wk_p.tile([128, 4, 16, 128], BF16)
                wv_bf = wv_p2.tile([128, 16, 512], BF16)
                # compute-critical first: K chain a waits only its wk slice,
                # and its first matmuls only the first hsT kt-group
                nc.sync.dma_start(hsT[:, 0, 0:4], hsT_d[:, 0, 0:4])
                nc.sync.dma_start(wk_bf[:, 0], wk_d[:, 0])
                for g in range(1, 4):
                    nc.sync.dma_start(hsT[:, 0, 4 * g:4 * (g + 1)],
                                      hsT_d[:, 0, 4 * g:4 * (g + 1)])
                for a in range(1, 4):
                    nc.sync.dma_start(wk_bf[:, a], wk_d[:, a])
                nc.sync.dma_start(rot_bf[:], rot_d[:])
                nc.sync.dma_start(c4k[:], c4k_d[:])
                nc.sync.dma_start(s4k[:], s4k_d[:])
                nc.sync.dma_start(wv_bf[:], wv_d[:])
                nc.sync.dma_start(hsT[:, 1], hsT_d[:, 1])
                nc.sync.dma_start(hsqT[:], hsqT_d[:])
                for a in range(4):
                    wq_bf = wq_p.tile([128, 16, 4, 128], BF16, tag="wqbf")
                    nc.sync.dma_start(wq_bf[:], wq_d[:, a])
                    wq_tiles.append(wq_bf)
                nc.sync.dma_start(masks_bf[:], masks_d[:])
                nc.sync.dma_start(c4q[:], c4q_d[:])
                nc.sync.dma_start(s4q[:], s4q_d[:])

                def k_chain(a, ch):
                    pk = projA.tile([128, 512], F32, tag="projA")
                    for kt in range(16):
                        nc.tensor.matmul(
                            pk[:], wk_bf[:, a, kt, :],
                            hsT[:, ch, kt, :],
                            start=(kt == 0), stop=(kt == 15))
                    flush()
                    rope_defer(pk, c4k, s4k, 512 * ch, 512,
                               kT[:, a, 512 * ch:512 * (ch + 1)], projA, "projA")

                def v_tile(tt):
                    pv32 = projA.tile([128, 512], F32, tag="projA")
                    for kt in range(16):
                        nc.tensor.matmul(
                            pv32[:], hsT[:, tt // 4, kt,
                                         128 * (tt % 4):128 * (tt % 4 + 1)],
                            wv_bf[:, kt, :], start=(kt == 0), stop=(kt == 15))
                    flush()
                    nc.vector.tensor_copy(
                        v_sb[:, tt, :, 0:64],
                        pv32.rearrange("p (g c) -> p g c", g=8))

                def q_chain(a, i, psum_pool, tag, pre_flush=True):
                    pq = psum_pool.tile([128, 512], F32, tag=tag)
                    for kt in range(16):
                        nc.tensor.matmul(
                            pq[:], wq_tiles[a][:, kt, i, :], hsqT[:, kt, :],
                            start=(kt == 0), stop=(kt == 15))
                    if pre_flush:
                        flush()
                    rope_defer(pq, c4q, s4q, 0, NQ, qT[:, 4 * a + i, :],
                               psum_pool, tag)

                for a in range(4):
                    k_chain(a, 0)
                for tt in range(4):
                    v_tile(tt)
                for a in range(4):
                    k_chain(a, 1)
                for tt in range(4, 8):
                    v_tile(tt)
                    if tt >= 6:
                        q_chain(0, tt - 6, projA, "projA")
                q_chain(0, 2, projA, "projA")
                q_chain(0, 3, projA, "projA")
                flush()

            # ---- phases B + C interleaved ----
            wo_p = pool(name="wobf", bufs=1)
            # wo lands in the SBUF space phase A freed (hsT/wk/wv)
            wo_bf = wo_p.tile([128, 16, D], BF16)
            nc.sync.dma_start(wo_bf[:], wo_d[:])
            osb_p = pool(name="osb", bufs=2)

            with ExitStack() as bctx:
                bpool = lambda *a, **k: bctx.enter_context(tc.tile_pool(*a, **k))
                projB = bpool(name="projB", bufs=2, space="PSUM")
                sc_ps = bpool(name="sc_ps", bufs=2, space="PSUM")
                pv_ps = bpool(name="pv_ps", bufs=2, space="PSUM")

                # Filler generators: the attention kb loop is ACT-bound
                # (exp ~1.1us vs ~0.65us of PE per iteration), and the PE
                # queue is in-order, so independent matmuls must be EMITTED
                # inside the kb loop to fill the stalls. Generators dribble
                # Q-projection / out-projection work one matmul at a time.
                def gen_q_chain(a, i):
                    pq = projB.tile([128, 512], F32, tag="projB")
                    for kt in range(16):
                        nc.tensor.matmul(
                            pq[:], wq_tiles[a][:, kt, i, :], hsqT[:, kt, :],
                            start=(kt == 0), stop=(kt == 15))
                        yield
                    x_sb = rope_p.tile([128, NQ], BF16, tag="rsb")
                    nc.vector.tensor_copy(x_sb[:], pq[:])
                    yield
                    pr = projB.tile([128, NQ], F32, tag="projB")
                    nc.tensor.matmul(pr[:], rot_bf[:], x_sb[:],
                                     start=True, stop=True)
                    yield
                    dst = qT[:, 4 * a + i, :]
                    t1 = rope_p.tile([128, NQ], BF16, tag="rt1")
                    nc.vector.tensor_tensor(t1[:], pr[:], s4q[:], MUL)
                    t2 = rope_p.tile([128, NQ], BF16, tag="rt2")
                    nc.vector.tensor_tensor(t2[:], x_sb[:], c4q[:], MUL)
                    nc.vector.tensor_tensor(dst, t1[:], t2[:], ADD)
                    yield

                def gen_out_chunk(j):
                    for oc in range(4):
                        po = projB.tile([128, 512], F32, tag="projB")
                        for cht in range(16):
                            nc.tensor.matmul(
                                po[:], attn_sb[:, cht, 128 * j:128 * (j + 1)],
                                wo_bf[:, cht, 512 * oc:512 * (oc + 1)],
                                start=(cht == 0), stop=(cht == 15))
                            yield
                        o_sb = osb_p.tile([128, 512], F32, tag="osb")
                        nc.vector.tensor_copy(o_sb[:], po[:])
                        nc.sync.dma_start(
                            out[128 * j:128 * (j + 1), 512 * oc:512 * (oc + 1)],
                            o_sb[:])
                        yield

                from collections import deque
                fillers = deque()

                def pull(n):
                    emitted = 0
                    while emitted < n and fillers:
                        try:
                            next(fillers[0])
                            emitted += 1
                        except StopIteration:
                            fillers.popleft()

                for a in range(4):
                    if a < 3:
                        for i in range(4):
                            fillers.append(gen_q_chain(a + 1, i))
                    # attention for kv pair a (both parities row-paired)
                    for j in range(4):
                        nkb = SCHED[j]
                        pv0 = pv_ps.tile([65, 4, 128], F32, tag="pv", name="pv0")
                        pv1 = pv_ps.tile([65, 4, 128], F32, tag="pv", name="pv1")
                        for kb in range(nkb):
                            sc = sc_ps.tile([128, 2, 4, 128], F32, tag="sc")
                            for h in range(2):
                                nc.tensor.matmul(
                                    sc[:, h],
                                    kT[64 * h:64 * (h + 1), a,
                                       128 * kb:128 * (kb + 1)],
                                    qT[64 * h:64 * (h + 1), 4 * a:4 * a + 4,
                                       128 * j:128 * (j + 1)],
                                    start=True, stop=True)
                            pull(1)
                            ex = exp_p.tile([128, 2, 4, 128], BF16, tag="ex")
                            nc.scalar.activation(ex[:], sc[:], AF.Exp, scale=SCALE)
                            if kb >= nkb - 2:
                                mk = masks_bf[:, j, kb - (nkb - 2)]
                                mkb = mk[:, None, None, :].to_broadcast(
                                    (128, 2, 4, 128))
                                nc.vector.tensor_tensor(ex[:], ex[:], mkb, MUL)
                            for h, pv in ((0, pv0), (1, pv1)):
                                nc.tensor.matmul(
                                    pv[:], v_sb[:, kb, 2 * a + h, :], ex[:, h],
                                    start=(kb == 0), stop=(kb == nkb - 1))
                            pull(1)
                        # early release: stage pv (unnormalized) + its ones-row
                        # into SBUF so the PSUM banks free after ~6 quick DVE
                        # ops, then normalize attn_sb in place off the PE path
                        l_sb = norm_p.tile([1, 2, 4, 128], F32, tag="lsb")
                        nc.vector.tensor_copy(l_sb[:, 0], pv0[64:65, :, :])
                        nc.vector.tensor_copy(l_sb[:, 1], pv1[64:65, :, :])
                        for par, pv in ((0, pv0), (1, pv1)):
                            for po_ in range(2):
                                nc.vector.tensor_copy(
                                    attn_sb[64 * po_:64 * (po_ + 1),
                                            4 * a + 2 * par:4 * a + 2 * par + 2,
                                            128 * j:128 * (j + 1)],
                                    pv[0:64, po_::2, :])
                        rc = norm_p.tile([1, 2, 4, 128], F32, tag="recip")
                        nc.vector.reciprocal_approx_fast(
                            rc.rearrange("p a b q -> p (a b q)"),
                            l_sb.rearrange("p a b q -> p (a b q)"))
                        rb = norm_p.tile([128, 2, 4, 128], F32, tag="rb")
                        nc.gpsimd.partition_broadcast(rb[:], rc[:])
                        for par in (0, 1):
                            for po_ in range(2):
                                sl = attn_sb[64 * po_:64 * (po_ + 1),
                                             4 * a + 2 * par:4 * a + 2 * par + 2,
                                             128 * j:128 * (j + 1)]
                                eng = nc.vector if po_ == 0 else nc.gpsimd
                                eng.tensor_tensor(
                                    sl, sl,
                                    rb[64 * po_:64 * (po_ + 1), par, po_::2, :],
                                    MUL)
                        if a == 3:
                            # attn_sb cols of chunk j now complete: its
                            # out-projection becomes available filler
                            fillers.append(gen_out_chunk(j))
                        pull(3)
                    # drain this group's fillers before the next needs them
                    if a < 3:
                        pull(1000)
                pull(1000)

    nc.finalize()
    return nc


def _core_chunks(c):
    return CHUNKS_EVEN if c % 2 == 0 else CHUNKS_ODD


def _host_consts():
    rot = np.zeros((128, 128), np.float32)
    for o in (0, 64):
        for d in range(32):
            rot[o + 32 + d, o + d] = -1.0
            rot[o + d, o + 32 + d] = 1.0
    return rot.astype(ml_dtypes.bfloat16)


def _to_bf16(x):
    return np.ascontiguousarray(x.astype(ml_dtypes.bfloat16))


_NC_CACHE = {}
_LAST_INMAPS = None


def kernel(hidden_states, cos, sin, Wq, Wk, Wv, Wo):
    hidden_states = np.asarray(hidden_states, dtype=np.float32)
    cos = np.asarray(cos, dtype=np.float32)
    sin = np.asarray(sin, dtype=np.float32)
    Wq = np.asarray(Wq, dtype=np.float32)
    Wk = np.asarray(Wk, dtype=np.float32)
    Wv = np.asarray(Wv, dtype=np.float32)
    Wo = np.asarray(Wo, dtype=np.float32)

    if "nc" not in _NC_CACHE:
        _NC_CACHE["nc"] = build_nc()
    nc = _NC_CACHE["nc"]

    rot = _host_consts()
    # weight SBUF layouts (shared by all cores)
    wk_sb = _to_bf16(Wk.reshape(16, 128, 4, 128).transpose(1, 2, 0, 3))
    wv_sb = _to_bf16(Wv.reshape(16, 128, 512).transpose(1, 0, 2))
    wo_sb = _to_bf16(Wo.reshape(16, 128, D).transpose(1, 0, 2))
    # Wq col = 512a + 256r + 64i + c  ->  [ki, a, kt, i, 64r + c]
    wq_sb = _to_bf16(Wq.reshape(16, 128, 4, 2, 4, 64)
                     .transpose(1, 2, 0, 4, 3, 5).reshape(128, 4, 16, 4, 128))
    # rope tables: positions restart per sequence, so one table serves all
    cs_seq = cos[:S]     # [S, 32]
    sn_seq = sin[:S]
    c4k = _to_bf16(np.tile(cs_seq.T, (4, 1)))       # [128, S]
    s4k = _to_bf16(np.tile(sn_seq.T, (4, 1)))

    in_maps = []
    for c in range(8):
        p = c // 2
        chunks = _core_chunks(c)
        rows_rel = np.concatenate([np.arange(128 * cj, 128 * (cj + 1))
                                   for cj in chunks])
        hs_seq = hidden_states[p * S:(p + 1) * S]               # [S, D]
        hsT = hs_seq.T.reshape(16, 128, S).transpose(1, 0, 2)   # [128,16,S]
        hsT2 = np.stack([hsT[:, :, :512], hsT[:, :, 512:]], axis=1)
        hs_q = hs_seq[rows_rel]                                 # [NQ, D]
        hsqT = hs_q.T.reshape(16, 128, NQ).transpose(1, 0, 2)
        c4q = np.ascontiguousarray(c4k[:, rows_rel])
        s4q = np.ascontiguousarray(s4k[:, rows_rel])
        masks = np.zeros((128, 4, 2, 128), np.float32)
        for j in range(4):
            cj = chunks[j]
            for m in range(2):
                kb = SCHED[j] - 2 + m
                qabs = 128 * cj + np.arange(128)
                kabs = 128 * kb + np.arange(128)
                masks[:, j, m, :] = (qabs[None, :] >= kabs[:, None])
        in_maps.append(dict(
            hsT=_to_bf16(hsT2), hsqT=_to_bf16(hsqT),
            wk=wk_sb, wv=wv_sb, wq=wq_sb, wo=wo_sb,
            c4k=c4k, s4k=s4k, c4q=c4q, s4q=s4q,
            rot=rot, masks=masks.astype(ml_dtypes.bfloat16),
        ))

    global _LAST_INMAPS
    _LAST_INMAPS = in_maps

    last_err = None
    for _attempt in range(2):
        try:
            res = run_bass_kernel_spmd(nc, in_maps, core_ids=list(range(8)))
            break
        except Exception as e:  # one retry: device occasionally needs a reset
            last_err = e
    else:
        raise last_err

    outp = np.zeros((B * S, D), np.float32)
    for c in range(8):
        p = c // 2
        chunks = _core_chunks(c)
        rows_rel = np.concatenate([np.arange(128 * cj, 128 * (cj + 1))
                                   for cj in chunks])
        outp[p * S + rows_rel] = res.results[c]["out"]
    return outp


# revision 23
# speedup vs baseline: 1.3976x; 1.3976x over previous
"""Exaone GQA flash-attention block on 8 Trainium2 NeuronCores.

Sharding: core pair (2p, 2p+1) handles prefill sequence p (S=1024). Within a
pair, the 8 causal 128-token q-chunks are split {0,2,5,7} / {1,3,4,6} so the
per-chunk key-block counts {1,3,6,8} vs {2,4,5,7} both fit under the uniform
compile-time schedule (2,4,6,8) with only 2 wasted blocks per core. No
cross-core communication: every core produces final output rows for its own
512 q-tokens; the host concatenates.

All layout work happens on the host (numpy, free): hs is transposed, weights
are rearranged into their exact SBUF layouts, everything is cast to bf16, and
rope tables / causal masks are prebuilt. The device program is pure DMA-in ->
matmul chains -> attention -> matmul chains -> DMA-out:

  phase A: kT = Wk^T @ hsT (rope via a +-1 rotation matmul), V natural
           [tok, ch] with an appended ones column so the PV matmul also
           produces the softmax denominator
  phase B (per kv-pair a): Q chains + rope for the 4 slots of group a, then
           attention: scores per (par, q-chunk, key-block) as two row-paired
           K=64 matmuls (partitions 0-63 / 64-127 run concurrently in
           different PE row groups), one fused exp over both parities on ACT,
           multiplicative masks only on the last two schedule positions,
           PV accumulated in PSUM [65, 4, 128]; normalization via the
           broadcast reciprocal of the ones-row
  phase C: out = attn^T.T @ Wo streamed per 512-wide output chunk.
"""
import sys
sys.path.insert(0, '/opt/trn_rl_repo')

from contextlib import ExitStack

import ml_dtypes
import numpy as np

import concourse.bass as bass
import concourse.mybir as mybir
import concourse.tile as tile
from concourse import bacc
from concourse.bass_utils import run_bass_kernel_spmd

F32 = mybir.dt.float32
BF16 = mybir.dt.bfloat16
AF = mybir.ActivationFunctionType
MUL = mybir.AluOpType.mult
ADD = mybir.AluOpType.add

B, S, D = 4, 1024, 2048
HQ, HKV, HD = 32, 8, 64
SCALE = HD ** -0.5
NQ = 512                       # q tokens per core
SCHED = (2, 4, 6, 8)           # key blocks per schedule slot (uniform)
CHUNKS_EVEN = (0, 2, 5, 7)     # q-chunk of schedule slot j, even cores
CHUNKS_ODD = (1, 3, 4, 6)


def build_nc():
    nc = bacc.Bacc("TRN2", target_bir_lowering=False, debug=False,
                   num_devices=8, num_swdge_queues=4)

    hsT_d = nc.dram_tensor("hsT", [128, 2, 16, 512], BF16, kind="ExternalInput")
    hsqT_d = nc.dram_tensor("hsqT", [128, 16, NQ], BF16, kind="ExternalInput")
    wk_d = nc.dram_tensor("wk", [128, 4, 16, 128], BF16, kind="ExternalInput")
    wv_d = nc.dram_tensor("wv", [128, 16, 512], BF16, kind="ExternalInput")
    wq_d = nc.dram_tensor("wq", [128, 4, 16, 4, 128], BF16, kind="ExternalInput")
    wo_d = nc.dram_tensor("wo", [128, 16, D], BF16, kind="ExternalInput")
    c4k_d = nc.dram_tensor("c4k", [128, S], BF16, kind="ExternalInput")
    s4k_d = nc.dram_tensor("s4k", [128, S], BF16, kind="ExternalInput")
    c4q_d = nc.dram_tensor("c4q", [128, NQ], BF16, kind="ExternalInput")
    s4q_d = nc.dram_tensor("s4q", [128, NQ], BF16, kind="ExternalInput")
    rot_d = nc.dram_tensor("rot", [128, 128], BF16, kind="ExternalInput")
    masks_d = nc.dram_tensor("masks", [128, 4, 2, 128], BF16, kind="ExternalInput")
    out = nc.dram_tensor("out", [NQ, D], F32, kind="ExternalOutput")

    with tile.TileContext(nc) as tc:
        with ExitStack() as ctx:
            pool = lambda *a, **k: ctx.enter_context(tc.tile_pool(*a, **k))
            qT_p = pool(name="qT", bufs=1)
            kT_p = pool(name="kT", bufs=1)
            v_p = pool(name="vsb", bufs=1)
            attn_p = pool(name="attn", bufs=1)
            const_p = pool(name="const", bufs=1)
            exp_p = pool(name="exps", bufs=6)
            rope_p = pool(name="rope", bufs=2)
            norm_p = pool(name="norm", bufs=1)

            qT = qT_p.tile([128, 16, NQ], BF16)
            kT = kT_p.tile([128, 4, S], BF16)
            v_sb = v_p.tile([128, 8, 8, 65], BF16)
            attn_sb = attn_p.tile([128, 16, NQ], BF16)

            rot_bf = const_p.tile([128, 128], BF16)
            masks_bf = const_p.tile([128, 4, 2, 128], BF16)
            c4k = const_p.tile([128, S], BF16)
            s4k = const_p.tile([128, S], BF16)
            c4q = const_p.tile([128, NQ], BF16)
            s4q = const_p.tile([128, NQ], BF16)

            nc.vector.memset(v_sb[:, :, :, 64], 1.0)

            # rope in two halves so the rot matmul never heads the PE queue
            # before its x_sb copy is ready: the finish part is emitted after
            # the NEXT chain's matmuls (deferred via `pending`).
            pending = []

            def flush():
                for f in pending:
                    f()
                pending.clear()

            def rope_defer(psum, c4, s4, col0, n, dst, psum_pool, tag):
                x_sb = rope_p.tile([128, n], BF16, tag="rsb")
                nc.vector.tensor_copy(x_sb[:], psum[:])

                def fin():
                    pr = psum_pool.tile([128, n], F32, tag=tag)
                    nc.tensor.matmul(pr[:], rot_bf[:], x_sb[:],
                                     start=True, stop=True)
                    t1 = rope_p.tile([128, n], BF16, tag="rt1")
                    nc.vector.tensor_tensor(t1[:], pr[:], s4[:, col0:col0 + n],
                                            MUL)
                    t2 = rope_p.tile([128, n], BF16, tag="rt2")
                    nc.vector.tensor_tensor(t2[:], x_sb[:], c4[:, col0:col0 + n],
                                            MUL)
                    nc.vector.tensor_tensor(dst, t1[:], t2[:], ADD)
                pending.append(fin)

            hsq_p = pool(name="hsq", bufs=1)
            wq_p = pool(name="wqbf", bufs=2)
            hsqT = hsq_p.tile([128, 16, NQ], BF16)
            wq_tiles = []

            # ---- phase A: K/V projections + Q chains for group 0 ----
            with ExitStack() as actx:
                apool = lambda *a, **k: actx.enter_context(tc.tile_pool(*a, **k))
                hsT_p = apool(name="hsT", bufs=1)
                wk_p = apool(name="wkbf", bufs=1)
                wv_p2 = apool(name="wvbf", bufs=1)
                projA = apool(name="projA", bufs=6, space="PSUM")

                hsT = hsT_p.tile([128, 2, 16, 512], BF16)
                wk_bf = wk_p.tile([128, 4, 16, 128], BF16)
                wv_bf = wv_p2.tile([128, 16, 512], BF16)
                # compute-critical first: K chain a waits only its wk slice,
                # and its first matmuls only the first hsT kt-group
                nc.sync.dma_start(hsT[:, 0, 0:4], hsT_d[:, 0, 0:4])
                nc.sync.dma_start(wk_bf[:, 0], wk_d[:, 0])
                for g in range(1, 4):
                    nc.sync.dma_start(hsT[:, 0, 4 * g:4 * (g + 1)],
                                      hsT_d[:, 0, 4 * g:4 * (g + 1)])
                for a in range(1, 4):
                    nc.sync.dma_start(wk_bf[:, a], wk_d[:, a])
                nc.sync.dma_start(rot_bf[:], rot_d[:])
                nc.sync.dma_start(c4k[:], c4k_d[:])
                nc.sync.dma_start(s4k[:], s4k_d[:])
                nc.sync.dma_start(wv_bf[:], wv_d[:])
                nc.sync.dma_start(hsT[:, 1], hsT_d[:, 1])
                nc.sync.dma_start(hsqT[:], hsqT_d[:])
                for a in range(4):
                    wq_bf = wq_p.tile([128, 16, 4, 128], BF16, tag="wqbf")
                    nc.sync.dma_start(wq_bf[:], wq_d[:, a])
                    wq_tiles.append(wq_bf)
                nc.sync.dma_start(masks_bf[:], masks_d[:])
                nc.sync.dma_start(c4q[:], c4q_d[:])
                nc.sync.dma_start(s4q[:], s4q_d[:])

                def k_chain(a, ch):
                    pk = projA.tile([128, 512], F32, tag="projA")
                    for kt in range(16):
                        nc.tensor.matmul(
                            pk[:], wk_bf[:, a, kt, :],
                            hsT[:, ch, kt, :],
                            start=(kt == 0), stop=(kt == 15))
                    flush()
                    rope_defer(pk, c4k, s4k, 512 * ch, 512,
                               kT[:, a, 512 * ch:512 * (ch + 1)], projA, "projA")

                def v_tile(tt):
                    pv32 = projA.tile([128, 512], F32, tag="projA")
                    for kt in range(16):
                        nc.tensor.matmul(
                            pv32[:], hsT[:, tt // 4, kt,
                                         128 * (tt % 4):128 * (tt % 4 + 1)],
                            wv_bf[:, kt, :], start=(kt == 0), stop=(kt == 15))
                    flush()
                    nc.vector.tensor_copy(
                        v_sb[:, tt, :, 0:64],
                        pv32.rearrange("p (g c) -> p g c", g=8))

                def q_chain(a, i, psum_pool, tag, pre_flush=True):
                    pq = psum_pool.tile([128, 512], F32, tag=tag)
                    for kt in range(16):
                        nc.tensor.matmul(
                            pq[:], wq_tiles[a][:, kt, i, :], hsqT[:, kt, :],
                            start=(kt == 0), stop=(kt == 15))
                    if pre_flush:
                        flush()
                    rope_defer(pq, c4q, s4q, 0, NQ, qT[:, 4 * a + i, :],
                               psum_pool, tag)

                for a in range(4):
                    k_chain(a, 0)
                for tt in range(4):
                    v_tile(tt)
                for a in range(4):
                    k_chain(a, 1)
                for tt in range(4, 8):
                    v_tile(tt)
                    if tt >= 6:
                        q_chain(0, tt - 6, projA, "projA")
                q_chain(0, 2, projA, "projA")
                q_chain(0, 3, projA, "projA")
                flush()

            # ---- phases B + C interleaved ----
            wo_p = pool(name="wobf", bufs=1)
            # wo lands in the SBUF space phase A freed (hsT/wk/wv)
            wo_bf = wo_p.tile([128, 16, D], BF16)
            nc.sync.dma_start(wo_bf[:], wo_d[:])
            osb_p = pool(name="osb", bufs=2)

            with ExitStack() as bctx:
                bpool = lambda *a, **k: bctx.enter_context(tc.tile_pool(*a, **k))
                projB = bpool(name="projB", bufs=2, space="PSUM")
                sc_ps = bpool(name="sc_ps", bufs=2, space="PSUM")
                pv_ps = bpool(name="pv_ps", bufs=2, space="PSUM")

                # Filler generators: the attention kb loop is ACT-bound
                # (exp ~1.1us vs ~0.65us of PE per iteration), and the PE
                # queue is in-order, so independent matmuls must be EMITTED
                # inside the kb loop to fill the stalls. Generators dribble
                # Q-projection / out-projection work one matmul at a time.
                def gen_q_chain(a, i):
                    pq = projB.tile([128, 512], F32, tag="projB")
                    for kt in range(16):
                        nc.tensor.matmul(
                            pq[:], wq_tiles[a][:, kt, i, :], hsqT[:, kt, :],
                            start=(kt == 0), stop=(kt == 15))
                        yield
                    x_sb = rope_p.tile([128, NQ], BF16, tag="rsb")
                    nc.vector.tensor_copy(x_sb[:], pq[:])
                    yield
                    pr = projB.tile([128, NQ], F32, tag="projB")
                    nc.tensor.matmul(pr[:], rot_bf[:], x_sb[:],
                                     start=True, stop=True)
                    yield
                    dst = qT[:, 4 * a + i, :]
                    t1 = rope_p.tile([128, NQ], BF16, tag="rt1")
                    nc.vector.tensor_tensor(t1[:], pr[:], s4q[:], MUL)
                    t2 = rope_p.tile([128, NQ], BF16, tag="rt2")
                    nc.vector.tensor_tensor(t2[:], x_sb[:], c4q[:], MUL)
                    nc.vector.tensor_tensor(dst, t1[:], t2[:], ADD)
                    yield

                def gen_out_chunk(j):
                    for oc in range(4):
                        po = projB.tile([128, 512], F32, tag="projB")
                        for cht in range(16):
                            nc.tensor.matmul(
                                po[:], attn_sb[:, cht, 128 * j:128 * (j + 1)],
                                wo_bf[:, cht, 512 * oc:512 * (oc + 1)],
                                start=(cht == 0), stop=(cht == 15))
                            yield
                        o_sb = osb_p.tile([128, 512], F32, tag="osb")
                        nc.vector.tensor_copy(o_sb[:], po[:])
                        nc.sync.dma_start(
                            out[128 * j:128 * (j + 1), 512 * oc:512 * (oc + 1)],
                            o_sb[:])
                        yield

                from collections import deque
                fillers = deque()

                def pull(n):
                    emitted = 0
                    while emitted < n and fillers:
                        try:
                            next(fillers[0])
                            emitted += 1
                        except StopIteration:
                            fillers.popleft()

                for a in range(4):
                    if a < 3:
                        for i in range(4):
                            fillers.append(gen_q_chain(a + 1, i))
                    # attention for kv pair a (both parities row-paired)
                    for j in range(4):
                        nkb = SCHED[j]
                        pv0 = pv_ps.tile([65, 4, 128], F32, tag="pv", name="pv0")
                        pv1 = pv_ps.tile([65, 4, 128], F32, tag="pv", name="pv1")
                        for kb in range(nkb):
                            sc = sc_ps.tile([128, 2, 4, 128], F32, tag="sc")
                            for h in range(2):
                                nc.tensor.matmul(
                                    sc[:, h],
                                    kT[64 * h:64 * (h + 1), a,
                                       128 * kb:128 * (kb + 1)],
                                    qT[64 * h:64 * (h + 1), 4 * a:4 * a + 4,
                                       128 * j:128 * (j + 1)],
                                    start=True, stop=True)
                            pull(1)
                            ex = exp_p.tile([128, 2, 4, 128], BF16, tag="ex")
                            nc.scalar.activation(ex[:], sc[:], AF.Exp, scale=SCALE)
                            if kb >= nkb - 2:
                                mk = masks_bf[:, j, kb - (nkb - 2)]
                                mkb = mk[:, None, None, :].to_broadcast(
                                    (128, 2, 4, 128))
                                nc.vector.tensor_tensor(ex[:], ex[:], mkb, MUL)
                            for h, pv in ((0, pv0), (1, pv1)):
                                nc.tensor.matmul(
                                    pv[:], v_sb[:, kb, 2 * a + h, :], ex[:, h],
                                    start=(kb == 0), stop=(kb == nkb - 1))
                            pull(1)
                        # normalize via reciprocal of the ones-row
                        l_sb = norm_p.tile([1, 2, 4, 128], F32, tag="lsb")
                        nc.vector.tensor_copy(l_sb[:, 0], pv0[64:65, :, :])
                        nc.vector.tensor_copy(l_sb[:, 1], pv1[64:65, :, :])
                        rc = norm_p.tile([1, 2, 4, 128], F32, tag="recip")
                        nc.vector.reciprocal_approx_fast(
                            rc.rearrange("p a b q -> p (a b q)"),
                            l_sb.rearrange("p a b q -> p (a b q)"))
                        rb = norm_p.tile([64, 2, 4, 128], F32, tag="rb")
                        nc.gpsimd.partition_broadcast(rb[:], rc[:])
                        for par, pv in ((0, pv0), (1, pv1)):
                            for po_ in range(2):
                                nc.vector.tensor_tensor(
                                    attn_sb[64 * po_:64 * (po_ + 1),
                                            4 * a + 2 * par:4 * a + 2 * par + 2,
                                            128 * j:128 * (j + 1)],
                                    pv[0:64, po_::2, :],
                                    rb[:, par, po_::2, :], MUL)
                        if a == 3:
                            # attn_sb cols of chunk j now complete: its
                            # out-projection becomes available filler
                            fillers.append(gen_out_chunk(j))
                        pull(6)
                    # drain this group's fillers before the next needs them
                    if a < 3:
                        pull(1000)
                pull(1000)

    nc.finalize()
    return nc


def _core_chunks(c):
    return CHUNKS_EVEN if c % 2 == 0 else CHUNKS_ODD


def _host_consts():
    rot = np.zeros((128, 128), np.float32)
    for o in (0, 64):
        for d in range(32):
            rot[o + 32 + d, o + d] = -1.0
            rot[o + d, o + 32 + d] = 1.0
    return rot.astype(ml_dtypes.bfloat16)


def _to_bf16(x):
    return np.ascontiguousarray(x.astype(ml_dtypes.bfloat16))


_NC_CACHE = {}
_LAST_INMAPS = None


def kernel(hidden_states, cos, sin, Wq, Wk, Wv, Wo):
    hidden_states = np.asarray(hidden_states, dtype=np.float32)
    cos = np.asarray(cos, dtype=np.float32)
    sin = np.asarray(sin, dtype=np.float32)
    Wq = np.asarray(Wq, dtype=np.float32)
    Wk = np.asarray(Wk, dtype=np.float32)
    Wv = np.asarray(Wv, dtype=np.float32)
    Wo = np.asarray(Wo, dtype=np.float32)

    if "nc" not in _NC_CACHE:
        _NC_CACHE["nc"] = build_nc()
    nc = _NC_CACHE["nc"]

    rot = _host_consts()
    # weight SBUF layouts (shared by all cores)
    wk_sb = _to_bf16(Wk.reshape(16, 128, 4, 128).transpose(1, 2, 0, 3))
    wv_sb = _to_bf16(Wv.reshape(16, 128, 512).transpose(1, 0, 2))
    wo_sb = _to_bf16(Wo.reshape(16, 128, D).transpose(1, 0, 2))
    # Wq col = 512a + 256r + 64i + c  ->  [ki, a, kt, i, 64r + c]
    wq_sb = _to_bf16(Wq.reshape(16, 128, 4, 2, 4, 64)
                     .transpose(1, 2, 0, 4, 3, 5).reshape(128, 4, 16, 4, 128))
    # rope tables: positions restart per sequence, so one table serves all
    cs_seq = cos[:S]     # [S, 32]
    sn_seq = sin[:S]
    c4k = _to_bf16(np.tile(cs_seq.T, (4, 1)))       # [128, S]
    s4k = _to_bf16(np.tile(sn_seq.T, (4, 1)))

    in_maps = []
    for c in range(8):
        p = c // 2
        chunks = _core_chunks(c)
        rows_rel = np.concatenate([np.arange(128 * cj, 128 * (cj + 1))
                                   for cj in chunks])
        hs_seq = hidden_states[p * S:(p + 1) * S]               # [S, D]
        hsT = hs_seq.T.reshape(16, 128, S).transpose(1, 0, 2)   # [128,16,S]
        hsT2 = np.stack([hsT[:, :, :512], hsT[:, :, 512:]], axis=1)
        hs_q = hs_seq[rows_rel]                                 # [NQ, D]
        hsqT = hs_q.T.reshape(16, 128, NQ).transpose(1, 0, 2)
        c4q = np.ascontiguousarray(c4k[:, rows_rel])
        s4q = np.ascontiguousarray(s4k[:, rows_rel])
        masks = np.zeros((128, 4, 2, 128), np.float32)
        for j in range(4):
            cj = chunks[j]
            for m in range(2):
                kb = SCHED[j] - 2 + m
                qabs = 128 * cj + np.arange(128)
                kabs = 128 * kb + np.arange(128)
                masks[:, j, m, :] = (qabs[None, :] >= kabs[:, None])
        in_maps.append(dict(
            hsT=_to_bf16(hsT2), hsqT=_to_bf16(hsqT),
            wk=wk_sb, wv=wv_sb, wq=wq_sb, wo=wo_sb,
            c4k=c4k, s4k=s4k, c4q=c4q, s4q=s4q,
            rot=rot, masks=masks.astype(ml_dtypes.bfloat16),
        ))

    global _LAST_INMAPS
    _LAST_INMAPS = in_maps

    last_err = None
    for _attempt in range(2):
        try:
            res = run_bass_kernel_spmd(nc, in_maps, core_ids=list(range(8)))
            break
        except Exception as e:  # one retry: device occasionally needs a reset
            last_err = e
    else:
        raise last_err

    outp = np.zeros((B * S, D), np.float32)
    for c in range(8):
        p = c // 2
        chunks = _core_chunks(c)
        rows_rel = np.concatenate([np.arange(128 * cj, 128 * (cj + 1))
                                   for cj in chunks])
        outp[p * S + rows_rel] = res.results[c]["out"]
    return outp
